# revision 23
# baseline (speedup 1.0000x reference)
"""AttnBlock (GroupNorm -> 1x1 qkv conv -> full attention -> 1x1 proj -> residual)
for x[8, 256, 64, 64] fp32, data-parallel over batch on 8 NeuronCores.

v3: fp8e4m3 DoubleRow matmuls for QKV, scores and PV (2x PE throughput), with
the algebra folded so fp8 never touches the residual path:
  - GroupNorm rides the mandatory x->fp8 cast: x8 = ACT(x, Identity,
    scale=alpha, bias=beta) per channel chunk. No weight scaling, no
    on-device bias corrections.
  - Host folds: pb' = proj_b + proj_w @ bv (since sum(attn)=1); bk dropped
    (per-query constant cancels in softmax); bq rides the q psum->fp8 cast.
  - proj bias pb' is pre-added to the residual prefill (out <- x + pb'), so
    per-block proj results DMA-accumulate straight from PSUM.
  - Scores stay at natural scale (sigma~16); exp on ACT applies scale=1/16,
    bias=-4 and writes fp8 e tiles that feed DoubleRow PV directly.
    exp(s/16-4) <= ~e^4.1 ~ 60 < 240 (fp8e4 max), no row max needed.
  - Z = sum_k e: pairwise e adds (Pool+DVE) then a running bf16 chain on DVE,
    partition-reduced by a ones-matmul reading the bf16 acc directly;
    1/Z via reciprocal_approx_fast; broadcast by a ones-matmul (f32r via
    bitcast). Epilogue of block b is injected across block b+1's pair stream
    with enough slack for the chain latency.
  - PSUM: 2x [P,2,512] score tiles (4 banks) + 4x [P,512] PV accumulators
    (2 generations) = 8 banks; zsum/zbcast/proj borrow score slots briefly.
"""

import contextlib
import ctypes
import os
import sys
import types

import numpy as np

import concourse.tile as tile
from concourse import bacc, mybir
from concourse.bass_utils import run_bass_kernel_spmd


def _ensure_ntff_hook() -> bool:
    """Install an antenv.axon_hooks shim backed by libaxon_pjrt.so so that
    run_bass_kernel_spmd(trace=True) can capture NTFF profiles under axon.
    Returns True when tracing is possible."""
    try:
        from antenv.axon_hooks import get_axon_ntff_profile_hook  # noqa: F401

        return True
    except ImportError:
        pass
    so_path = "/opt/axon/libaxon_pjrt.so"
    if not os.path.exists(so_path):
        return False
    try:
        lib = ctypes.CDLL(so_path)
        if not hasattr(lib, "axon_start_nrt_profile"):
            return False
        lib.axon_start_nrt_profile.argtypes = [
            ctypes.POINTER(ctypes.c_int64),
            ctypes.c_size_t,
        ]
        lib.axon_start_nrt_profile.restype = ctypes.c_int64
        lib.axon_stop_nrt_profile.argtypes = [ctypes.c_char_p]
        lib.axon_stop_nrt_profile.restype = ctypes.c_int64
    except OSError:
        return False

    @contextlib.contextmanager
    def _hook(output_dir, device_ids):
        import jax

        jax.devices()
        if device_ids:
            ids = (ctypes.c_int64 * len(device_ids))(*device_ids)
            rc = lib.axon_start_nrt_profile(ids, len(device_ids))
        else:
            rc = lib.axon_start_nrt_profile(None, 0)
        if rc != 0:
            raise RuntimeError(f"axon_start_nrt_profile rc={rc}")
        try:
            yield
        finally:
            n = lib.axon_stop_nrt_profile(str(output_dir).encode())
            print(f"profile: {n} file(s) written to {output_dir}", file=sys.stderr)

    mod = types.ModuleType("antenv.axon_hooks")
    _state = {"hook": _hook}
    mod.get_axon_ntff_profile_hook = lambda: _state["hook"]
    mod.set_axon_ntff_profile_hook = lambda h: _state.__setitem__("hook", h)
    sys.modules["antenv.axon_hooks"] = mod
    import antenv

    antenv.axon_hooks = mod
    return True

F32 = mybir.dt.float32
F32R = mybir.dt.float32r
BF16 = mybir.dt.bfloat16
F8 = mybir.dt.float8e4
AX = mybir.AluOpType
AF = mybir.ActivationFunctionType
DR = mybir.MatmulPerfMode.DoubleRow

C = 256          # channels
N = 4096         # tokens (64*64)
P = 128          # partitions
CO = 2           # channel chunks (C // P)
QB = 512         # queries per block
NQB = N // QB    # 8 query blocks
NKC = N // P     # 32 key chunks
NPR = NKC // 2   # 16 key chunk pairs (DoubleRow contracts 256 keys)
EPS = 1e-5

_LAST_RESULTS = None


def _build_program():
    nc = bacc.Bacc("TRN2", target_bir_lowering=False, debug=False, num_devices=8)

    x_d = nc.dram_tensor("x", [C, N], F32, kind="ExternalInput").ap()
    wqkT_d = nc.dram_tensor("wqkT", [C, 3 * C], F32, kind="ExternalInput").ap()
    bq_d = nc.dram_tensor("bq", [C], F32, kind="ExternalInput").ap()
    projT_d = nc.dram_tensor("projT", [C, C], F32, kind="ExternalInput").ap()
    pb_d = nc.dram_tensor("pb", [C], F32, kind="ExternalInput").ap()
    nw_d = nc.dram_tensor("nw", [C], F32, kind="ExternalInput").ap()
    nb_d = nc.dram_tensor("nb", [C], F32, kind="ExternalInput").ap()
    gh_d = nc.dram_tensor("ghmat", [P, P], F32, kind="ExternalInput").ap()
    out_d = nc.dram_tensor("out", [C, N], F32, kind="ExternalOutput").ap()

    # channel c = o*128 + p  ->  [partition, chunk, free]
    x_v = x_d.rearrange("(o p) m -> p o m", p=P)
    wqkT_v = wqkT_d.rearrange("(o p) m -> p o m", p=P)
    projT_v = projT_d.rearrange("(o p) m -> p o m", p=P)
    out_v = out_d.rearrange("(o p) m -> p o m", p=P)

    with tile.TileContext(nc) as tc:
        with (
            tc.tile_pool(name="cpool", bufs=1) as cpool,
            tc.tile_pool(name="bigs", bufs=1) as bigs,
            tc.tile_pool(name="spool", bufs=1) as spool,
            tc.tile_pool(name="epool", bufs=6) as epool,
            tc.tile_pool(name="t1pool", bufs=4) as t1pool,
            tc.tile_pool(name="accpool", bufs=3) as accpool,
            tc.tile_pool(name="zpool", bufs=2) as zpool,
            tc.tile_pool(name="wpool", bufs=2) as wpool,
            tc.tile_pool(name="psA", bufs=2, space="PSUM") as psA,
            tc.tile_pool(name="psO", bufs=4, space="PSUM") as psO,
        ):
            # ---- input loads: x split into 8 chunks so stats overlap the DMA ----
            x_sb = bigs.tile([P, CO, N], F32)
            for co in range(CO):
                for c in range(4):
                    csl = slice(c * 1024, (c + 1) * 1024)
                    nc.sync.dma_start(out=x_sb[:, co, csl], in_=x_v[:, co, csl])
            wqk_sb = cpool.tile([P, CO, 3 * C], F32)
            nc.sync.dma_start(out=wqk_sb, in_=wqkT_v)
            projT_sb = cpool.tile([P, CO, C], F32)
            nc.sync.dma_start(out=projT_sb, in_=projT_v)

            def vec_tile(name, d_ap):
                t = cpool.tile([P, CO], F32, name=name)
                nc.sync.dma_start(out=t, in_=d_ap.rearrange("(o p) -> p o", p=P))
                return t

            bq_sb = vec_tile("bq_sb", bq_d)
            pb_sb = vec_tile("pb_sb", pb_d)
            nw_sb = vec_tile("nw_sb", nw_d)
            nb_sb = vec_tile("nb_sb", nb_d)
            gh_sb = cpool.tile([P, P], F32)
            nc.sync.dma_start(out=gh_sb, in_=gh_d)
            onesf = cpool.tile([P, 2, P], F32)
            nc.vector.memset(onesf, 1.0)
            ones8 = cpool.tile([P, 2, P], F8)
            nc.vector.tensor_copy(out=ones8, in_=onesf)
            eps_t = cpool.tile([P, 1], F32)
            nc.vector.memset(eps_t, EPS)
            neg4_t = cpool.tile([P, 1], F32)
            nc.vector.memset(neg4_t, -4.0)

            with nc.allow_low_precision(reason="fp8 attention path"):
                # preload the exp table set while the x DMA streams in
                dummy8 = cpool.tile([P, 1], F8)
                nc.scalar.activation(out=dummy8, in_=eps_t, func=AF.Exp)
                # static weight quantization on Pool (overlaps the x DMA and
                # stays off the DVE stats critical path)
                wqk8 = cpool.tile([P, CO, 3 * C], F8)
                for co in range(CO):
                    nc.gpsimd.tensor_copy(out=wqk8[:, co, :], in_=wqk_sb[:, co, :])
                projT8 = cpool.tile([P, CO, C], F8)
                nc.gpsimd.tensor_copy(out=projT8, in_=projT_sb)

                # ---- GroupNorm stats (per-channel along free axis) ----
                stats = spool.tile([P, CO, 8, 6], F32)
                mv = spool.tile([P, CO, 2], F32)
                for co in range(CO):
                    for s in range(8):
                        nc.vector.bn_stats(
                            out=stats[:, co, s, :],
                            in_=x_sb[:, co, s * 512 : (s + 1) * 512],
                        )
                    nc.vector.bn_aggr(out=mv[:, co, :], in_=stats[:, co])
                # rstats cols: [mean_co0, mean_co1, ex2_co0, ex2_co1]
                rstats = spool.tile([P, 4], F32)
                nc.vector.tensor_copy(out=rstats[:, 0:2], in_=mv[:, :, 0])
                nc.vector.tensor_tensor(
                    out=rstats[:, 2:4], in0=mv[:, :, 0], in1=mv[:, :, 0], op=AX.mult)
                nc.vector.tensor_tensor(
                    out=rstats[:, 2:4], in0=rstats[:, 2:4], in1=mv[:, :, 1], op=AX.add)
                # group mean over 8 adjacent partitions, broadcast back, in one
                # block-diagonal (1/8) indicator matmul (fp32 exact)
                bps = psA.tile([P, 2, QB], F32, tag="spair", name="bps")
                nc.tensor.matmul(bps[:, 0, 0:4], lhsT=gh_sb, rhs=rstats,
                                 start=True, stop=True)
                bss = spool.tile([P, 4], F32)
                nc.vector.tensor_copy(out=bss, in_=bps[:, 0, 0:4])
                # var = ex2 - mu^2 ; rstd = 1/sqrt(var + eps)
                var = spool.tile([P, 2], F32)
                nc.vector.tensor_tensor(
                    out=var, in0=bss[:, 0:2], in1=bss[:, 0:2], op=AX.mult)
                nc.vector.tensor_tensor(
                    out=var, in0=bss[:, 2:4], in1=var, op=AX.subtract)
                sd = spool.tile([P, 2], F32)
                nc.scalar.activation(out=sd, in_=var, func=AF.Sqrt, bias=eps_t, scale=1.0)
                rstd = spool.tile([P, 2], F32)
                nc.vector.reciprocal(out=rstd, in_=sd)
                alpha = spool.tile([P, 2], F32)
                nc.vector.tensor_tensor(out=alpha, in0=rstd, in1=nw_sb, op=AX.mult)
                beta = spool.tile([P, 2], F32)
                nc.vector.tensor_tensor(out=beta, in0=bss[:, 0:2], in1=alpha, op=AX.mult)
                nc.vector.tensor_tensor(out=beta, in0=nb_sb, in1=beta, op=AX.subtract)

                # residual prefill with the proj bias folded in:
                # out <- x + pb'; per-block proj results DMA-accumulate later
                xpb = bigs.tile([P, CO, N], F32)
                for co in range(CO):
                    for c in range(4):
                        csl = slice(c * 1024, (c + 1) * 1024)
                        nc.vector.tensor_scalar(
                            out=xpb[:, co, csl], in0=x_sb[:, co, csl],
                            scalar1=pb_sb[:, co : co + 1], scalar2=None, op0=AX.add)
                    nc.sync.dma_start(out=out_v[:, co, :], in_=xpb[:, co, :])

                # ---- QKV (DoubleRow fp8); x8 = groupnormed x via the cast ----
                x8 = bigs.tile([P, CO, N], F8)
                q8 = bigs.tile([P, CO, N], F8)
                k8 = bigs.tile([P, CO, N], F8)
                vt8 = bigs.tile([P, NKC, C], F8)
                out8 = bigs.tile([P, CO, N], F8)

                for blk in range(NQB):
                    sl = slice(blk * QB, (blk + 1) * QB)
                    if blk % 2 == 0:
                        # groupnorm rides the x->fp8 cast; DVE 2x_2P makes this
                        # far cheaper than an ACT activation
                        dsl = slice(blk * QB, (blk + 2) * QB)
                        for co in range(CO):
                            nc.vector.tensor_scalar(
                                out=x8[:, co, dsl], in0=x_sb[:, co, dsl],
                                scalar1=alpha[:, co : co + 1],
                                scalar2=beta[:, co : co + 1],
                                op0=AX.mult, op1=AX.add)
                    qk_ps = psA.tile([P, 2, QB], F32, tag="spair", name="qk_ps")
                    for cout in range(CO):
                        nc.tensor.matmul(
                            qk_ps[:, cout, :],
                            lhsT=wqk8[:, :, cout * P : (cout + 1) * P],
                            rhs=x8[:, :, sl],
                            start=True, stop=True, perf_mode=DR)
                        nc.scalar.activation(
                            out=q8[:, cout, sl], in_=qk_ps[:, cout, :],
                            func=AF.Identity, bias=bq_sb[:, cout : cout + 1], scale=1.0)
                    kk_ps = psA.tile([P, 2, QB], F32, tag="spair", name="kk_ps")
                    for cout in range(CO):
                        nc.tensor.matmul(
                            kk_ps[:, cout, :],
                            lhsT=wqk8[:, :, C + cout * P : C + (cout + 1) * P],
                            rhs=x8[:, :, sl],
                            start=True, stop=True, perf_mode=DR)
                        if cout == 0:
                            nc.vector.tensor_copy(
                                out=k8[:, cout, sl], in_=kk_ps[:, cout, :])
                        else:
                            nc.scalar.copy(out=k8[:, cout, sl], in_=kk_ps[:, cout, :])
                    # vt pairs: 4 token chunks -> 2 psum pair tiles
                    for kp in range(2 * blk, 2 * blk + 2):
                        vt_ps = psA.tile([P, 2, QB], F32, tag="spair", name="vt_ps")
                        for i in range(2):
                            ko = 2 * kp + i
                            nc.tensor.matmul(
                                vt_ps[:, i, 0:C],
                                lhsT=x8[:, :, ko * P : (ko + 1) * P],
                                rhs=wqk8[:, :, 2 * C : 3 * C],
                                start=True, stop=True, perf_mode=DR)
                        if kp % 2 == 0:
                            nc.vector.tensor_copy(
                                out=vt8[:, 2 * kp : 2 * kp + 2, :], in_=vt_ps[:, :, 0:C])
                        else:
                            nc.scalar.copy(
                                out=vt8[:, 2 * kp : 2 * kp + 2, :], in_=vt_ps[:, :, 0:C])

                # ---- attention (pipelined; prev block epilogue injected) ----
                def make_block(qb):
                    ctx = {"qb": qb}
                    ctx["pso"] = [
                        psO.tile([P, QB], F32, tag="psout", name=f"pso{cc}")
                        for cc in range(CO)
                    ]
                    ctx["es"] = [None] * NPR
                    ctx["t1"] = [None] * 8
                    return ctx

                def do_s(ctx, j):
                    qb = ctx["qb"]
                    ps = psA.tile([P, 2, QB], F32, tag="spair", name="s_ps")
                    for i in range(2):
                        kc = 2 * j + i
                        nc.tensor.matmul(
                            ps[:, i, :],
                            lhsT=k8[:, :, kc * P : (kc + 1) * P],
                            rhs=q8[:, :, qb * QB : (qb + 1) * QB],
                            start=True, stop=True, perf_mode=DR)
                    e = epool.tile([P, 2, QB], F8, name="e_tile")
                    nc.scalar.activation(
                        out=e, in_=ps, func=AF.Exp, bias=neg4_t, scale=1.0 / 16.0)
                    ctx["es"][j] = e

                def do_tree(ctx, j):
                    # pairwise e adds (Pool for the early ones, DVE later) feed a
                    # running bf16 chain so only one add trails the last exp
                    if j % 2 == 1:
                        i = j // 2
                        eng = nc.gpsimd if i < 4 else nc.vector
                        t = t1pool.tile([P, 2, QB], BF16, name="t1")
                        eng.tensor_tensor(
                            out=t, in0=ctx["es"][2 * i], in1=ctx["es"][2 * i + 1],
                            op=AX.add)
                        ctx["t1"][i] = t
                        if i >= 1:
                            # final add emits fp8 so the Z partition-reduce can
                            # be a DoubleRow ones-matmul (folds the pair dim too)
                            dt = F8 if i == 7 else BF16
                            acc = accpool.tile([P, 2, QB], dt, name="acc")
                            prev_acc = ctx["t1"][0] if i == 1 else ctx["acc"]
                            nc.vector.tensor_tensor(
                                out=acc, in0=prev_acc, in1=t, op=AX.add)
                            ctx["acc"] = acc
                    if j == NPR - 1:
                        ctx["zacc"] = ctx["acc"]

                def do_pv(ctx, j):
                    for cc in range(CO):
                        nc.tensor.matmul(
                            ctx["pso"][cc],
                            lhsT=vt8[:, 2 * j : 2 * j + 2, cc * P : (cc + 1) * P],
                            rhs=ctx["es"][j],
                            start=(j == 0), stop=(j == NPR - 1), perf_mode=DR)

                def epi_zsum(ctx):
                    # ones lhsT makes every output partition the full key-sum:
                    # Z is reduced AND broadcast by this single matmul
                    zps = psA.tile([P, 2, QB], F32, tag="spair", name="zps")
                    nc.tensor.matmul(
                        zps[:, 0, :], lhsT=ones8, rhs=ctx["zacc"],
                        start=True, stop=True, perf_mode=DR)
                    ctx["zps"] = zps

                def epi_recip(ctx):
                    zbs = wpool.tile([P, QB], F32, name="zbs")
                    nc.vector.reciprocal_approx_fast(out=zbs, in_=ctx["zps"][:, 0, :])
                    ctx["zbs"] = zbs

                def epi_out(ctx, cc):
                    qb = ctx["qb"]
                    nc.vector.tensor_tensor(
                        out=out8[:, cc, qb * QB : (qb + 1) * QB],
                        in0=ctx["pso"][cc], in1=ctx["zbs"], op=AX.mult)

                def epi_proj(ctx, cout, last=False):
                    qb = ctx["qb"]
                    sl = slice(qb * QB, (qb + 1) * QB)
                    # proj psum borrows a score-pair slot (half per cout)
                    if cout == 0:
                        ctx["pjps"] = psA.tile(
                            [P, 2, QB], F32, tag="spair", name="pj_ps")
                    ps = ctx["pjps"][:, cout, :]
                    nc.tensor.matmul(
                        ps,
                        lhsT=projT8[:, :, cout * P : (cout + 1) * P],
                        rhs=out8[:, :, sl],
                        start=True, stop=True, perf_mode=DR)
                    if last:
                        # prefill is overwritten here: add bias + residual in one op
                        fin = wpool.tile([P, QB], F32, name="fin")
                        nc.vector.scalar_tensor_tensor(
                            out=fin, in0=ps, scalar=pb_sb[:, cout : cout + 1],
                            in1=x_sb[:, cout, sl], op0=AX.add, op1=AX.add)
                        nc.sync.dma_start(out=out_v[:, cout, sl], in_=fin)
                    else:
                        # pb' already sits in the prefill; accumulate raw proj
                        fin = wpool.tile([P, QB], F32, name="fin")
                        nc.vector.tensor_copy(out=fin, in_=ps)
                        nc.gpsimd.dma_start(
                            out=out_v[:, cout, sl], in_=fin, accum_op=AX.add)

                def inject(prev, j):
                    if prev is None:
                        return
                    if j == 3:
                        epi_zsum(prev)
                    elif j == 5:
                        epi_recip(prev)
                    elif j == 9:
                        epi_out(prev, 0)
                    elif j == 10:
                        epi_out(prev, 1)
                    elif j == 12:
                        epi_proj(prev, 0)
                    elif j == 14:
                        epi_proj(prev, 1)

                prev = None
                for qb in range(NQB):
                    ctx = make_block(qb)
                    do_s(ctx, 0)
                    do_s(ctx, 1)
                    do_tree(ctx, 1)
                    for j in range(2, NPR):
                        do_s(ctx, j)
                        do_pv(ctx, j - 2)
                        inject(prev, j - 2)
                        do_tree(ctx, j)
                    do_pv(ctx, NPR - 2)
                    inject(prev, NPR - 2)
                    do_pv(ctx, NPR - 1)
                    inject(prev, NPR - 1)
                    prev = ctx
                # tail: last block epilogue
                epi_zsum(prev)
                epi_recip(prev)
                epi_out(prev, 0)
                epi_out(prev, 1)
                epi_proj(prev, 0, last=True)
                epi_proj(prev, 1, last=True)

    nc.compile()
    return nc


def _host_inputs(x, norm_w, norm_b, qkv_w, qkv_b, proj_w, proj_b):
    f = np.float32
    wqkT = np.ascontiguousarray(qkv_w.T).astype(f)   # [c_in, 3C]
    bq = qkv_b[:C].astype(f)
    bv = qkv_b[2 * C : 3 * C].astype(f)
    projT = np.ascontiguousarray(proj_w.T).astype(f)
    # v bias folds into the proj bias because sum_k attn = 1
    pb = (proj_b + proj_w @ bv).astype(f)
    gh = np.zeros((P, P), f)
    gh[np.arange(P)[:, None] // 8 == np.arange(P)[None, :] // 8] = 0.125
    shared = {
        "wqkT": wqkT, "bq": bq,
        "projT": projT, "pb": pb,
        "nw": norm_w.astype(f), "nb": norm_b.astype(f),
        "ghmat": gh,
    }
    xs = np.ascontiguousarray(x.reshape(x.shape[0], C, N).astype(f))
    return [dict(shared, x=xs[i]) for i in range(x.shape[0])]


def kernel(x, norm_w, norm_b, qkv_w, qkv_b, proj_w, proj_b):
    global _LAST_RESULTS
    B = x.shape[0]
    nc = _build_program()
    in_maps = _host_inputs(x, norm_w, norm_b, qkv_w, qkv_b, proj_w, proj_b)
    trace = bool(int(os.environ.get("KERNEL_TRACE", "0"))) or bool(
        os.environ.get("BASS_TRACE")
    )
    if trace:
        trace = _ensure_ntff_hook()
    res = run_bass_kernel_spmd(
        nc, in_maps, core_ids=list(range(B)), trace=trace,
    )
    _LAST_RESULTS = res
    out = np.stack([res.results[i]["out"] for i in range(B)])
    return out.reshape(B, C, 64, 64)
